# revision 26
# baseline (speedup 1.0000x reference)
"""Trainium2 Bass kernel for relative-position attention (nn_AttentionMechanism).

Math (per batch b):
  q,k,v = h@Wq, h@Wk, h@Wv  (biases are zero in this problem)
  scores[l,r] = (q[l].k[r] + q[l].E[l-r+1023] + k[r].E[l-r+1023]) / sqrt(64)
  out = softmax(scores) @ v @ Wd

Sharding: 8 cores = (batch b in 0..3) x (query half lh in 0..1).
Each core computes out rows [lh*512, lh*512+512) for batch b.

Per-core algorithm (T orientation: score tiles are [r partitions, l free]):
  - host pre-transposes x; qT/kT = W^T @ xT matmuls in bf16 (scaled by SCALE);
    v natural with a 64-wide ones block appended (softmax denominators come out
    as extra rows of the PV matmul).
  - Relative-position shear E[l-r+1023]:
      rel_k^T[r,l] = KE[r, 127-r_in+l] within each 128-row J block: a
        per-partition byte-offset shift, done by an SBUF->SBUF DMA whose
        source access pattern has row stride (W-1) instead of W (the DGE
        walks SBUF linearly per descriptor).  KE tiles are fp16 staging
        copies of the PE outputs.
      rel_q^T needs a partition-crossing shear, which only the HWDGE xbar
        transpose can do: QEr (music-transformer layout, fp16) is written to
        DRAM with row stride 1536 and read back with row stride 1535 through
        the transpose DMA -> rel_q^T directly.
      rel_k is then DMA-accumulated (SWDGE accum_op=add) onto rel_q in SBUF.
  - content scores via PE (bf16, 1 cycle/row); the combined rel tile is added
    into the same PSUM by an identity-matmul accumulate; exp on ScalarE reads
    PSUM directly (no max subtraction needed: |scores| <~ 1.5 by construction).
  - PV + denominators on PE (fp32r), per-head normalize via broadcast
    reciprocal multiply, then the out-projection.
"""

import sys

sys.path.insert(0, "/opt/trn_rl_repo")

import numpy as np

import concourse.bass as bass
import concourse.mybir as mybir
import concourse.tile as tile
from concourse import bacc
from concourse.bass_utils import run_bass_kernel_spmd

FP32 = mybir.dt.float32
FP32R = mybir.dt.float32r
FP16 = mybir.dt.float16
BF16 = mybir.dt.bfloat16
ADD = mybir.AluOpType.add
MULT = mybir.AluOpType.mult
EXP = mybir.ActivationFunctionType.Exp


def _mm(nc, out, lhsT, rhs, **kw):
    nc.tensor.matmul(out, lhsT, rhs, **kw)

N_CORES = 8
D, H, HD = 768, 12, 64
LQ, LK = 512, 1024
EW = 1536          # E window rows per core (= LQ + LK + pad)
KD_W = 640         # KE chunk width (639 used + 1 pad col)
QD_W = 1536        # qd row stride in DRAM
SCALE = 0.35355339059327373  # 8**-0.5 applied to q,k AND E => all terms get /8


def _strided_view(ap, dims, extra_offset):
    """Return a copy of `ap` with its [step,count] pairs and offset replaced."""
    v = ap.copy()
    a = v.ap
    assert len(a) == len(dims), (a, dims)
    for i, d in enumerate(dims):
        a[i] = d
    v.ap = a
    v.offset = v.offset + extra_offset
    return v


def build_nc(repeats=1):
    nc = bacc.Bacc("TRN2", target_bir_lowering=False, debug=False,
                   num_devices=N_CORES)

    hq = nc.dram_tensor("hidden_q_T", [D, LQ], BF16, kind="ExternalInput").ap()
    hkv = nc.dram_tensor("hidden_kv_T", [D, LK], BF16, kind="ExternalInput").ap()
    wq = nc.dram_tensor("Wq", [D, D], BF16, kind="ExternalInput").ap()
    wk = nc.dram_tensor("Wk", [D, D], BF16, kind="ExternalInput").ap()
    wv = nc.dram_tensor("Wv", [D, D], BF16, kind="ExternalInput").ap()
    wd = nc.dram_tensor("Wd", [D, D], BF16, kind="ExternalInput").ap()
    demb = nc.dram_tensor("demb_win_T", [HD, EW], BF16, kind="ExternalInput").ap()
    dembr = nc.dram_tensor("demb_win_rev_T", [HD, EW], BF16, kind="ExternalInput").ap()
    ident = nc.dram_tensor("ident128", [128, 128], FP16, kind="ExternalInput").ap()
    out = nc.dram_tensor("out", [LQ, D], FP32, kind="ExternalOutput").ap()

    with tile.TileContext(nc) as tc:
        # one shared qd scratch: cross-repeat WAR hazards serialize repeats,
        # which the calibrated-delta timing in test.py depends on
        qd_dram = nc.dram_tensor("qd_scratch", [H, LQ, QD_W], FP16).ap()
        for r in range(repeats):
            _body(nc, tc, hq, hkv, wq, wk, wv, wd, demb, dembr, ident, out,
                  qd_dram)
    nc.compile()
    return nc


def _body(nc, tc, hq, hkv, wq, wk, wv, wd, demb, dembr, ident, out, qd_dram):
    with tc.tile_pool(name="const", bufs=1) as cp:
        ones_row = cp.tile([1, 64], BF16, tag="ones_row")
        nc.gpsimd.memset(ones_row[:, :], 1.0)
        eT = cp.tile([128, EW], BF16, tag="eT")    # rows 0:64 == 64:128 (replicated)
        erT = cp.tile([128, EW], BF16, tag="erT")
        idt = cp.tile([128, 128], FP16, tag="idt")
        kT = [cp.tile([128, LK], BF16, tag=f"kT{i}", name=f"kT{i}") for i in range(6)]
        qT = [cp.tile([128, LQ], BF16, tag=f"qT{i}", name=f"qT{i}") for i in range(6)]
        vv = [cp.tile([128, 780], BF16, tag=f"v{i}", name=f"v{i}") for i in range(8)]
        ctxT = [cp.tile([128, LQ], BF16, tag=f"ctxT{i}", name=f"ctxT{i}") for i in range(6)]

        # ---------------- Phase A+B: loads (host pre-transposed) + projections
        with tc.tile_pool(name="xt", bufs=1) as xp:
            xT = [xp.tile([128, LK], BF16, tag=f"xT{i}", name=f"xT{i}") for i in range(6)]
            xqT = [xp.tile([128, LQ], BF16, tag=f"xqT{i}", name=f"xqT{i}") for i in range(6)]

            nc.sync.dma_start(out=idt[:, :], in_=ident[:, :])
            for half in range(2):
                nc.sync.dma_start(out=eT[64 * half:64 * (half + 1), :], in_=demb[:, :])
                nc.sync.dma_start(out=erT[64 * half:64 * (half + 1), :], in_=dembr[:, :])
            for i in range(6):
                nc.sync.dma_start(out=xT[i][:, :], in_=hkv[128 * i:128 * (i + 1), :])
                nc.sync.dma_start(out=xqT[i][:, :], in_=hq[128 * i:128 * (i + 1), :])

            # projections
            with tc.tile_pool(name="wld", bufs=1) as wp, \
                 tc.tile_pool(name="psB", bufs=2, space="PSUM") as pb:
                for widx, (wdram, dst, rhs_tiles, n_tok) in enumerate((
                        (wk, kT, xT, LK), (wq, qT, xqT, LQ), (wv, None, xT, LK))):
                    wtiles = []
                    for kk in range(6):
                        wt = wp.tile([128, D], BF16, tag=f"w{kk}")
                        nc.sync.dma_start(out=wt[:, :], in_=wdram[128 * kk:128 * (kk + 1), :])
                        wtiles.append(wt)
                    if dst is not None:  # q/k: out is [D, n_tok] transposed, bf16
                        for m in range(6):
                            ps = pb.tile([128, LK], FP32, tag="projp")
                            for kk in range(6):
                                for nh in range(n_tok // 512):
                                    _mm(nc,
                                        ps[:, 512 * nh:512 * (nh + 1)],
                                        wtiles[kk][:, 128 * m:128 * (m + 1)],
                                        rhs_tiles[kk][:, 512 * nh:512 * (nh + 1)],
                                        start=(kk == 0), stop=(kk == 5))
                            nc.scalar.mul(dst[m][:, 0:n_tok], ps[:, 0:n_tok], SCALE)
                    else:  # v: natural [tok, D]
                        for r in range(8):
                            ps = pb.tile([128, D], FP32, tag="projp")
                            for kk in range(6):
                                for o, w in ((0, 512), (512, 256)):
                                    _mm(nc,
                                        ps[:, o:o + w],
                                        xT[kk][:, 128 * r:128 * (r + 1)],
                                        wtiles[kk][:, o:o + w],
                                        start=(kk == 0), stop=(kk == 5))
                            nc.gpsimd.memset(vv[r][:, :], 1.0)
                            vdst = vv[r][:, 0:D].rearrange("p (h e) -> p h e", e=64)
                            vdst = _strided_view(vdst, [vdst.ap[0], (65, 12), (1, 64)], 0)
                            nc.scalar.copy(vdst, ps[:, 0:D].rearrange(
                                "p (h e) -> p h e", e=64))

        # ---------------- Phase C: per-head attention ----------------
        # ke pool: fp16 KE staging tiles, 8 per head x 2-head lookahead
        with tc.tile_pool(name="psC", bufs=2, space="PSUM") as pc, \
             tc.tile_pool(name="psCS", bufs=3, space="PSUM") as pcs, \
             tc.tile_pool(name="psCTX", bufs=1, space="PSUM") as pctx, \
             tc.tile_pool(name="keP", bufs=32) as kep, \
             tc.tile_pool(name="wkC", bufs=3) as wc, \
             tc.tile_pool(name="relP", bufs=40) as relp, \
             tc.tile_pool(name="wkC3", bufs=12) as wc3:
            ke_tiles = {}   # (h, J) -> tile
            rel_tiles = {}  # (h, J) -> tile

            def _emit_writes(h):
                hc, hp = h // 2, h % 2
                hr = slice(64 * hp, 64 * (hp + 1))
                # KE chunks -> fp16 SBUF staging (no DRAM roundtrip)
                for J in range(8):
                    w0 = 896 - 128 * J
                    kdp = pc.tile([128, KD_W], FP32, tag="kdqd")
                    lhsT = kT[hc][hr, 128 * J:128 * (J + 1)]
                    _mm(nc, kdp[:, 0:512], lhsT, eT[hr, w0:w0 + 512],
                        start=True, stop=True)
                    _mm(nc, kdp[:, 512:KD_W], lhsT, eT[hr, w0 + 512:w0 + KD_W],
                        start=True, stop=True)
                    ke = kep.tile([128, KD_W], FP16, tag="ke")
                    # alternate staging copies between ACT and DVE
                    if J % 2 == 0:
                        nc.scalar.copy(ke[:, :], kdp[:, :])
                    else:
                        nc.vector.tensor_copy(ke[:, :], kdp[:, :])
                    ke_tiles[(h, J)] = ke

                # qd chunks -> DRAM (fp16, reversed window) for the xbar reads
                for Ip in range(2):
                    qd_sb = wc.tile([128, 2, 1152], FP16, tag="qd_sb")
                    for half in range(2):
                        I = 2 * Ip + half
                        c0 = 384 - 128 * I
                        lhsT = qT[hc][hr, 128 * I:128 * (I + 1)]
                        qdpA = pc.tile([128, KD_W], FP32, tag="kdqd")
                        for o, w in ((0, 512), (512, 128)):
                            _mm(nc, qdpA[:, o:o + w], lhsT,
                                erT[hr, c0 + o:c0 + o + w],
                                start=True, stop=True)
                        nc.vector.tensor_copy(qd_sb[:, half, 0:KD_W], qdpA[:, :])
                        qdpB = pc.tile([128, KD_W], FP32, tag="kdqd")
                        _mm(nc, qdpB[:, 0:512], lhsT,
                            erT[hr, c0 + KD_W:c0 + KD_W + 512],
                            start=True, stop=True)
                        nc.scalar.copy(qd_sb[:, half, KD_W:1152], qdpB[:, 0:512])
                    # rows of the I-pair: row step 1536, I-step = 128*1536 - 128
                    c0p = 384 - 256 * Ip
                    qdw = _strided_view(
                        qd_dram[h, 256 * Ip:256 * Ip + 128, c0p:c0p + 1152]
                        .unsqueeze(1),
                        [(QD_W, 128), (128 * QD_W - 128, 2), (1, 1152)], 0)
                    nc.sync.dma_start(out=qdw, in_=qd_sb[:, :, :].rearrange(
                        "p a b -> p (a b)").rearrange("p (a b) -> p a b", a=2))

            def _emit_rels(h):
                # rel reads for head h, one slot ahead of the consumer:
                # rel_q^T via xbar transpose from DRAM (SP sequencer, idle);
                # rel_k^T via on-chip diag DMA from the KE staging tiles.
                for J in range(8):
                    rq_sb = relp.tile([128, LQ], FP16, tag="rq_sb")
                    qdv = _strided_view(qd_dram[h], [(QD_W - 1, LQ), (1, 128)],
                                        512 + 128 * J)
                    nc.sync.dma_start(out=rq_sb[:, :], in_=qdv, transpose=True)
                    ke = ke_tiles.pop((h, J))
                    kev = _strided_view(ke[:, :], [(KD_W - 1, 128), (1, LQ)], 127)
                    nc.gpsimd.dma_start(out=rq_sb[:, :], in_=kev, accum_op=ADD)
                    rel_tiles[(h, J)] = rq_sb

            def _emit_scores(h):
                hc, hp = h // 2, h % 2
                hr = slice(64 * hp, 64 * (hp + 1))
                ctxp = pctx.tile([65, LQ], FP32, tag="ctxp")
                p_sbs = []
                for J in range(8):
                    rel_sb = rel_tiles.pop((h, J))
                    csp = pcs.tile([128, LQ], FP32, tag="csp")
                    nc.tensor.matmul(csp[:, :], kT[hc][hr, 128 * J:128 * (J + 1)],
                                     qT[hc][hr, :], start=True, stop=False)
                    nc.tensor.matmul(csp[:, :], idt[:, :], rel_sb[:, :],
                                     start=False, stop=True)
                    p_sb = wc3.tile([128, LQ], BF16, tag="p_sb")
                    nc.scalar.activation(p_sb[:, :], csp[:, :], EXP)
                    p_sbs.append(p_sb)
                for J in range(8):
                    # PV (rows 0:64) + denominators (row 64) share the rhs stream
                    _mm(nc, ctxp[:, :], vv[J][:, 65 * h:65 * h + 65],
                        p_sbs[J][:, :], start=(J == 0), stop=(J == 7))

                # normalize: ctxT_h = ctx' * (1/denom) broadcast over partitions
                recip = wc.tile([1, LQ], FP32, tag="recip")
                nc.vector.reciprocal(recip[:, :], ctxp[64:65, :])
                recb = wc.tile([1, LQ], BF16, tag="recb")
                nc.scalar.copy(recb[:, :], recip[:, :])
                bcp = pcs.tile([128, LQ], FP32, tag="csp")
                _mm(nc, bcp[0:64, :], ones_row[:, :], recb[:, :],
                    start=True, stop=True)
                bc_sb = wc.tile([64, LQ], FP32, tag="bc_sb")
                nc.scalar.copy(bc_sb[:, :], bcp[0:64, :])
                nc.vector.tensor_tensor(ctxT[hc][hr, :], ctxp[0:64, :],
                                        bc_sb[:, :], MULT)

            for h in range(H + 2):
                if 1 <= h <= H:
                    _emit_rels(h - 1)
                if h < H:
                    _emit_writes(h)
                if h >= 2:
                    _emit_scores(h - 2)

        # ---------------- Phase D: output projection ----------------
        with tc.tile_pool(name="wdld", bufs=1) as dp, \
             tc.tile_pool(name="psD", bufs=2, space="PSUM") as pd, \
             tc.tile_pool(name="oD", bufs=2) as od:
            wdt = []
            for kk in range(6):
                wt = dp.tile([128, D], BF16, tag=f"wd{kk}")
                nc.sync.dma_start(out=wt[:, :], in_=wd[128 * kk:128 * (kk + 1), :])
                wdt.append(wt)
            for lc in range(4):
                ps = pd.tile([128, D], FP32, tag="outp")
                for kk in range(6):
                    for o, w in ((0, 512), (512, 256)):
                        _mm(nc, ps[:, o:o + w],
                            ctxT[kk][:, 128 * lc:128 * (lc + 1)],
                            wdt[kk][:, o:o + w],
                            start=(kk == 0), stop=(kk == 5))
                o_sb = od.tile([128, D], FP32, tag="o_sb")
                nc.scalar.copy(o_sb[:, :], ps[:, :])
                nc.sync.dma_start(out=out[128 * lc:128 * (lc + 1), :], in_=o_sb[:, :])


_NC_CACHE = None


def _get_nc():
    global _NC_CACHE
    if _NC_CACHE is None:
        _NC_CACHE = build_nc()
    return _NC_CACHE


def make_in_maps(hidden_states, Wq, Wk, Wv, Wd, dist_emb):
    import ml_dtypes
    BF = ml_dtypes.bfloat16
    E = np.ascontiguousarray(np.asarray(dist_emb, np.float32))
    ident = np.eye(128, dtype=np.float16)
    Wqb = np.ascontiguousarray(np.asarray(Wq, np.float32).astype(BF))
    Wkb = np.ascontiguousarray(np.asarray(Wk, np.float32).astype(BF))
    Wvb = np.ascontiguousarray(np.asarray(Wv, np.float32).astype(BF))
    Wdb = np.ascontiguousarray(np.asarray(Wd, np.float32).astype(BF))
    in_maps = []
    for core in range(N_CORES):
        b, lh = core // 2, core % 2
        l0 = LQ * lh
        win = np.zeros((EW, HD), np.float32)
        n = min(EW, E.shape[0] - l0)
        win[:n] = E[l0:l0 + n]
        wins = (win * np.float32(SCALE)).astype(ml_dtypes.bfloat16)
        in_maps.append({
            "hidden_q_T": np.ascontiguousarray(
                np.asarray(hidden_states[b, l0:l0 + LQ].T, np.float32).astype(BF)),
            "hidden_kv_T": np.ascontiguousarray(
                np.asarray(hidden_states[b].T, np.float32).astype(BF)),
            "Wq": Wqb, "Wk": Wkb, "Wv": Wvb, "Wd": Wdb,
            "demb_win_T": np.ascontiguousarray(wins.T),
            "demb_win_rev_T": np.ascontiguousarray(wins[::-1].T),
            "ident128": ident,
        })
    return in_maps


def run(inputs, trace=False):
    """Returns (full_output [4,1024,768], BassKernelResults)."""
    nc = _get_nc()
    in_maps = make_in_maps(inputs["hidden_states"], inputs["Wq"], inputs["Wk"],
                           inputs["Wv"], inputs["Wd"], inputs["dist_emb"])
    res = run_bass_kernel_spmd(nc, in_maps, list(range(N_CORES)), trace=trace)
    full = np.zeros((4, LK, D), np.float32)
    for core in range(N_CORES):
        b, lh = core // 2, core % 2
        full[b, LQ * lh:LQ * (lh + 1)] = res.results[core]["out"]
    return full, res


def kernel(**inputs):
    full, _ = run(inputs, trace=False)
    return full


if __name__ == "__main__":
    # quick self-build check
    nc = build_nc()
    print("built ok")
